# revision 9
# baseline (speedup 1.0000x reference)
"""CapsNet ClassCaps (dynamic routing) Trainium2 kernel, 8-core SPMD.

Problem: u_hat[b,k,c,o] = sum_i W[k,c,o,i] * x[b,k,i]; 3 routing iterations
(softmax over classes, weighted sum over capsules, squash, agreement).
B=32, K=1024, C=64, O=32, I=32.

Sharding: K split 8 ways (128 capsules/core). Each core:
  - streams its W shard [128k, 64c, 32o, 32i] (host pre-laid-out as
    [(k%4)*32+i, (k//4)*2048 + c*32 + o], fp32),
  - computes u_hat with fp32 PE matmuls packed 4-per-array via tile_position
    (contraction i=32, 4 capsules concurrently on diagonal 32x32 tiles),
  - keeps u_hat SBUF-resident in fp16, layout [p=(k%4,b), f=(k//4,c,o)],
  - runs routing on DVE (fused via fp16 tree reductions), with one 256KB
    AllReduce of the partial s_j per routing step (3 total),
  - emits its c_ij shard; v_j is replicated (taken from core 0).
"""
import numpy as np
import concourse.bass as bass
import concourse.mybir as mybir
import concourse.tile as tile
from concourse import bass_utils
from concourse.bacc import Bacc

dt = mybir.dt
f32, f16 = dt.float32, dt.float16

B, K, C, O, I = 32, 1024, 64, 32, 32
NCORES = 8
KSH = K // NCORES          # 128 capsules per core
G = KSH // 4               # 32 k-groups of 4
CO = C * O                 # 2048
FREE = G * CO              # 65536
GC = G * C                 # 2048

TREE_L1 = 4096             # tree tile layout offsets (fp16 elements)
_OFF = [0, 4096, 6144, 7168, 7680]


def _tree_outer(nc, tree, src3, red, inner, final_out_ap):
    """sum over outermost axis: src3 is an AP view [p, red, inner] (fp16).
    Halve `red` repeatedly; intermediate levels live in `tree` (fp16),
    final add writes fp32 `final_out_ap` [p, inner]."""
    add = mybir.AluOpType.add
    cur, level = src3, 0
    while red > 2:
        half = red // 2
        out = tree[:, _OFF[level]:_OFF[level] + half * inner] \
            .rearrange("p (r m) -> p r m", r=half, m=inner)
        nc.vector.tensor_tensor(out, cur[:, :half, :], cur[:, half:red, :], add)
        cur, red, level = out, half, level + 1
    nc.vector.tensor_tensor(final_out_ap, cur[:, 0, :], cur[:, 1, :], add)


def _tree_inner(nc, tree, src3, outer, red, final_out_ap):
    """sum over innermost axis: src3 is an AP view [p, outer, red] (fp16).
    Final add writes fp32 `final_out_ap` [p, outer]."""
    add = mybir.AluOpType.add
    cur, level = src3, 0
    while red > 2:
        half = red // 2
        out = tree[:, _OFF[level]:_OFF[level] + outer * half] \
            .rearrange("p (m r) -> p m r", m=outer, r=half)
        nc.vector.tensor_tensor(out, cur[:, :, :half], cur[:, :, half:red], add)
        cur, red, level = out, half, level + 1
    nc.vector.tensor_tensor(final_out_ap, cur[:, :, 0], cur[:, :, 1], add)


def _kernel_body(nc, tc, xT_d, sel_d, w_d, v_d, cij_d):
    add = mybir.AluOpType.add
    mult = mybir.AluOpType.mult
    sub = mybir.AluOpType.subtract
    mx_op = mybir.AluOpType.max

    with tc.tile_pool(name="u", bufs=1) as up, \
         tc.tile_pool(name="x", bufs=1) as xp, \
         tc.tile_pool(name="wst", bufs=2) as wp, \
         tc.tile_pool(name="work", bufs=2) as wkp, \
         tc.tile_pool(name="small", bufs=1) as sp, \
         tc.tile_pool(name="ps", bufs=4, space="PSUM") as pp, \
         tc.tile_pool(name="dram", bufs=1, space="DRAM") as dp:

        u = up.tile([128, FREE], f16)
        x_sb = xp.tile([128, G * B], f32)
        sel_sb = xp.tile([128, 2 * B], f32)
        nc.sync.dma_start(x_sb[:], xT_d)
        nc.sync.dma_start(sel_sb[:], sel_d)

        bij = sp.tile([128, GC], f32)
        a_e = sp.tile([128, GC], f32)      # agreement, then exp, then cij fp32
        cw16 = sp.tile([128, GC], f16)
        mxz = sp.tile([128, G], f32)       # softmax max, then Z
        zi = sp.tile([128, G], f32)
        sP = sp.tile([128, CO], f32)       # partial s; [0:32] doubles as s/v
        vrep = sp.tile([128, CO], f16)
        sq = sp.tile([B, C], f32)
        tA = sp.tile([B, C], f32)
        tB = sp.tile([B, C], f32)

        # ---------------- production: u_hat matmuls ----------------
        for hg in range(2 * G):
            g, h = hg // 2, hg % 2
            wt = wp.tile([128, CO // 2], f32, tag="wst")
            nc.sync.dma_start(wt[:], w_d[:, hg * (CO // 2):(hg + 1) * (CO // 2)])
            for nch2 in range(2):
                nch = 2 * h + nch2
                ps = pp.tile([128, 512], f32, tag="pp")
                for kg in range(4):
                    nc.tensor.matmul(
                        ps[32 * kg:32 * kg + 32, :],
                        x_sb[32 * kg:32 * kg + 32, g * B:(g + 1) * B],
                        wt[32 * kg:32 * kg + 32, 512 * nch2:512 * (nch2 + 1)],
                        start=True, stop=True,
                        tile_position=(32 * kg, 32 * kg))
                nc.vector.tensor_copy(
                    u[:, g * CO + 512 * nch: g * CO + 512 * (nch + 1)], ps[:])

        # u viewed [p, g, c, o] / [p, g, co]
        u_gco = u[:].rearrange("p (g c o) -> p g c o", g=G, c=C, o=O)
        u_gf = u[:].rearrange("p (g f) -> p g f", g=G, f=CO)

        # ---------------- iter-0 s: s0 = sum_k u / C ----------------
        NCHK = 8
        CW = CO // NCHK  # 256
        for j in range(NCHK):
            tree = wkp.tile([128, 8192], f16, tag="work")
            src = u_gf[:, :, j * CW:(j + 1) * CW]
            _tree_outer(nc, tree, src, G, CW, sP[:, j * CW:(j + 1) * CW])

        def kg_fold_scale_ar(scaled, tag):
            # fold 4 kg partition groups via PE: s[b,co] = sum_p sel[p,b]*sP[p,co]
            lhs = sel_sb[:, B:2 * B] if scaled else sel_sb[:, 0:B]
            arin = dp.tile([B, CO], f32, tag=f"arin{tag}")
            arout = dp.tile([B, CO], f32, tag=f"arout{tag}")
            for q in range(4):
                fp = pp.tile([B, 512], f32, tag="fold")
                nc.tensor.matmul(fp[:], lhs, sP[:, 512 * q:512 * (q + 1)],
                                 start=True, stop=True)
                nc.vector.tensor_copy(sP[0:32, 512 * q:512 * (q + 1)], fp[:])
            nc.sync.dma_start(arin[:], sP[0:32, :])
            nc.gpsimd.collective_compute(
                "AllReduce", add,
                ins=[arin[:]], outs=[arout[:]],
                replica_groups=[list(range(NCORES))])
            nc.sync.dma_start(sP[0:32, :], arout[:])

        def squash():
            # sP[0:32] holds allreduced s [B, CO]; in-place -> v
            sv = sP[0:32, :]
            wk = wkp.tile([128, 8192], f16, tag="work")
            scr = wk[:].bitcast(f32)[0:32, 0:CO]
            nc.vector.tensor_tensor(scr, sv, sv, mult)
            s2 = scr.rearrange("p (c o) -> p c o", c=C, o=O)
            nc.vector.tensor_reduce(sq[:], s2, axis=mybir.AxisListType.X, op=add)
            nc.vector.tensor_scalar_add(tA[:], sq[:], 1.0)
            nc.vector.reciprocal(tA[:], tA[:])
            nc.vector.tensor_tensor(tA[:], sq[:], tA[:], mult)   # sq/(1+sq)
            nc.vector.tensor_scalar_add(tB[:], sq[:], 1e-8)
            nc.scalar.sqrt(tB[:], tB[:])
            nc.vector.reciprocal(tB[:], tB[:])
            nc.vector.tensor_tensor(tA[:], tA[:], tB[:], mult)   # full factor
            fbc = tA[:].unsqueeze(2).broadcast_to([B, C, O])
            svv = sv.rearrange("p (c o) -> p c o", c=C, o=O)
            nc.vector.tensor_tensor(svv, svv, fbc, mult)

        def build_vrep():
            nc.vector.tensor_copy(vrep[0:32, :], sP[0:32, :])
            for kg in range(1, 4):
                nc.sync.dma_start(vrep[32 * kg:32 * kg + 32, :], vrep[0:32, :])

        kg_fold_scale_ar(True, "0")
        squash()
        build_vrep()

        # ---------------- routing iterations ----------------
        for it in (0, 1):
            # agreement a[p,(g,c)] = sum_o u * vrep, in g-chunks of 4
            GCH = 4
            for j in range(G // GCH):
                am = wkp.tile([128, 8192], f16, tag="work")
                uv = u_gco[:, j * GCH:(j + 1) * GCH, :, :]
                vv = vrep[:].rearrange("p (c o) -> p c o", c=C, o=O) \
                            .unsqueeze(1).broadcast_to([128, GCH, C, O])
                nc.vector.tensor_tensor(
                    am[:].rearrange("p (g c o) -> p g c o", g=GCH, c=C, o=O),
                    uv, vv, mult)
                tree = wkp.tile([128, 8192], f16, tag="work")
                amv = am[:].rearrange("p (gc o) -> p gc o", gc=GCH * C, o=O)
                _tree_inner(nc, tree, amv, GCH * C, O,
                            a_e[:, j * GCH * C:(j + 1) * GCH * C])

            # b update
            if it == 0:
                nc.vector.tensor_copy(bij[:], a_e[:])
            else:
                nc.vector.tensor_tensor(bij[:], bij[:], a_e[:], add)

            # softmax over c: bij [p,(g,c)]
            bv = bij[:].rearrange("p (g c) -> p g c", g=G, c=C)
            nc.vector.tensor_reduce(mxz[:], bv, axis=mybir.AxisListType.X, op=mx_op)
            mbc = mxz[:].unsqueeze(2).broadcast_to([128, G, C])
            ev = a_e[:].rearrange("p (g c) -> p g c", g=G, c=C)
            nc.vector.tensor_tensor(ev, bv, mbc, sub)
            nc.scalar.activation(a_e[:], a_e[:], mybir.ActivationFunctionType.Exp)
            nc.vector.tensor_reduce(mxz[:], ev, axis=mybir.AxisListType.X, op=add)
            nc.vector.reciprocal(zi[:], mxz[:])
            zbc = zi[:].unsqueeze(2).broadcast_to([128, G, C])
            cv16 = cw16[:].rearrange("p (g c) -> p g c", g=G, c=C)
            nc.vector.tensor_tensor(cv16, ev, zbc, mult)
            if it == 1:
                # final c_ij output (fp32), then DMA out
                nc.vector.tensor_tensor(ev, ev, zbc, mult)
                nc.sync.dma_start(cij_d, a_e[:])

            # s pass: sP[p,(c,o)] = sum_g cw16 * u   (co-chunks of CW)
            for j in range(NCHK):
                sm = wkp.tile([128, 8192], f16, tag="work")
                c0 = j * (C // NCHK)
                c1 = (j + 1) * (C // NCHK)
                uvv = u_gco[:, :, c0:c1, :]
                cbc = cw16[:].rearrange("p (g c) -> p g c", g=G, c=C)[:, :, c0:c1] \
                             .unsqueeze(3).broadcast_to([128, G, C // NCHK, O])
                smv = sm[:].rearrange("p (g f) -> p g f", g=G, f=CW)
                nc.vector.tensor_tensor(
                    smv.rearrange("p g (c o) -> p g c o", c=C // NCHK, o=O),
                    uvv, cbc, mult)
                tree = wkp.tile([128, 8192], f16, tag="work")
                _tree_outer(nc, tree, smv, G, CW, sP[:, j * CW:(j + 1) * CW])

            kg_fold_scale_ar(False, str(it + 1))
            squash()
            if it == 0:
                build_vrep()
            else:
                nc.sync.dma_start(v_d, sP[0:32, :])


_NC_CACHE = {}


def _build():
    if "nc" in _NC_CACHE:
        return _NC_CACHE["nc"]
    nc = Bacc("TRN2", target_bir_lowering=False, debug=False,
              num_devices=NCORES)
    xT_d = nc.dram_tensor("xT", [128, G * B], f32, kind="ExternalInput").ap()
    sel_d = nc.dram_tensor("sel", [128, 2 * B], f32, kind="ExternalInput").ap()
    w_d = nc.dram_tensor("w", [128, FREE], f32, kind="ExternalInput").ap()
    v_d = nc.dram_tensor("v_out", [B, CO], f32, kind="ExternalOutput").ap()
    cij_d = nc.dram_tensor("cij_out", [128, GC], f32, kind="ExternalOutput").ap()
    with tile.TileContext(nc) as tc:
        _kernel_body(nc, tc, xT_d, sel_d, w_d, v_d, cij_d)
    nc.compile()
    _NC_CACHE["nc"] = nc
    return nc


def _shard_inputs(x, W):
    """Host-side shard + layout. Returns in_maps for the 8 cores."""
    in_maps = []
    for r in range(NCORES):
        k0 = r * KSH
        Wr = W[k0:k0 + KSH]                       # [128, C, O, I]
        # -> [(kg, i), (g, c, o)]
        Wr = Wr.reshape(G, 4, C, O, I).transpose(1, 4, 0, 2, 3)
        w_host = np.ascontiguousarray(Wr.reshape(128, FREE), dtype=np.float32)
        xr = x[:, k0:k0 + KSH, :]                 # [B, 128, I]
        xr = xr.reshape(B, G, 4, I).transpose(2, 3, 1, 0)   # [kg, i, g, b]
        x_host = np.ascontiguousarray(xr.reshape(128, G * B), dtype=np.float32)
        sel = np.zeros((128, 2 * B), dtype=np.float32)
        for kg in range(4):
            sel[kg * B:(kg + 1) * B, 0:B] = np.eye(B, dtype=np.float32)
            sel[kg * B:(kg + 1) * B, B:2 * B] = np.eye(B, dtype=np.float32) / C
        in_maps.append({"xT": x_host, "w": w_host, "sel": sel})
    return in_maps


def kernel(x, W, _profile=False):
    x = np.asarray(x, dtype=np.float32)
    W = np.asarray(W, dtype=np.float32)
    nc = _build()
    in_maps = _shard_inputs(x, W)
    res = bass_utils.run_bass_kernel_spmd(
        nc, in_maps, core_ids=list(range(NCORES)), trace=_profile)
    v = res.results[0]["v_out"].reshape(B, C, O)
    cs = []
    for r in range(NCORES):
        cr = res.results[r]["cij_out"].reshape(4, B, G, C)  # [kg, b, g, c]
        cs.append(cr.transpose(1, 2, 0, 3).reshape(B, KSH, C))  # [b,(g,kg),c]
    c_ij = np.concatenate(cs, axis=1)             # [B, K, C]
    if _profile:
        return (v, c_ij), res
    return v, c_ij


# revision 11
# speedup vs baseline: 1.0168x; 1.0168x over previous
"""CapsNet ClassCaps (dynamic routing) Trainium2 kernel, 8-core SPMD.

Problem: u_hat[b,k,c,o] = sum_i W[k,c,o,i] * x[b,k,i]; 3 routing iterations
(softmax over classes, weighted sum over capsules, squash, agreement).
B=32, K=1024, C=64, O=32, I=32.

Sharding: K split 8 ways (128 capsules/core). Each core:
  - streams its W shard [128k, 64c, 32o, 32i] (host pre-laid-out as
    [(k%4)*32+i, (k//4)*2048 + c*32 + o], fp32),
  - computes u_hat with fp32 PE matmuls packed 4-per-array via tile_position
    (contraction i=32, 4 capsules concurrently on diagonal 32x32 tiles),
  - keeps u_hat SBUF-resident in fp16, layout [p=(k%4,b), f=(k//4,c,o)],
  - runs routing on DVE (fused via fp16 tree reductions), with one 256KB
    AllReduce of the partial s_j per routing step (3 total),
  - emits its c_ij shard; v_j is replicated (taken from core 0).
"""
import numpy as np
import concourse.bass as bass
import concourse.mybir as mybir
import concourse.tile as tile
from concourse import bass_utils
from concourse.bacc import Bacc

dt = mybir.dt
f32, f16 = dt.float32, dt.float16

B, K, C, O, I = 32, 1024, 64, 32, 32
NCORES = 8
KSH = K // NCORES          # 128 capsules per core
G = KSH // 4               # 32 k-groups of 4
CO = C * O                 # 2048
FREE = G * CO              # 65536
GC = G * C                 # 2048

_OFF = [0, 2048, 3072, 3584]       # tree tile level offsets (fp16 elements)


def _tree_last(nc, tree, src3, pre, red, final_out_ap):
    """sum over innermost axis of [p, pre, red] (fp16, contiguous views).
    Final add writes fp32 `final_out_ap` [p, pre]."""
    add = mybir.AluOpType.add
    cur, level = src3, 0
    while red > 2:
        half = red // 2
        out = tree[:, _OFF[level]:_OFF[level] + pre * half] \
            .rearrange("p (m r) -> p m r", m=pre, r=half)
        nc.vector.tensor_tensor(out, cur[:, :, :half], cur[:, :, half:red], add)
        cur, red, level = out, half, level + 1
    nc.vector.tensor_tensor(final_out_ap, cur[:, :, 0], cur[:, :, 1], add)


def _tree_mid(nc, tree, src4, a, red, bk, final_out_ap):
    """sum over middle axis of [p, a, red, bk] (fp16). Final add writes fp32
    `final_out_ap` [p, a, bk]."""
    add = mybir.AluOpType.add
    cur, level = src4, 0
    while red > 2:
        half = red // 2
        out = tree[:, _OFF[level]:_OFF[level] + a * half * bk] \
            .rearrange("p (m r n) -> p m r n", m=a, r=half, n=bk)
        nc.vector.tensor_tensor(out, cur[:, :, :half, :], cur[:, :, half:red, :], add)
        cur, red, level = out, half, level + 1
    nc.vector.tensor_tensor(final_out_ap, cur[:, :, 0, :], cur[:, :, 1, :], add)


def _kernel_body(nc, tc, xT_d, sel_d, w_d, v_d, cij_d):
    add = mybir.AluOpType.add
    mult = mybir.AluOpType.mult
    sub = mybir.AluOpType.subtract
    mx_op = mybir.AluOpType.max

    with tc.tile_pool(name="u", bufs=1) as up, \
         tc.tile_pool(name="x", bufs=1) as xp, \
         tc.tile_pool(name="wst", bufs=2) as wp, \
         tc.tile_pool(name="work", bufs=2) as wkp, \
         tc.tile_pool(name="small", bufs=1) as sp, \
         tc.tile_pool(name="ps", bufs=4, space="PSUM") as pp, \
         tc.tile_pool(name="dram", bufs=1, space="DRAM") as dp:

        u = up.tile([128, FREE], f16)          # layout [p, (c, o, g)]
        x_sb = xp.tile([128, G * B], f32)
        sel_sb = xp.tile([128, 2 * B], f32)
        nc.sync.dma_start(x_sb[:], xT_d)
        nc.sync.dma_start(sel_sb[:], sel_d)

        bij = sp.tile([128, GC], f32)          # layout (c, g)
        a_e = sp.tile([128, GC], f32)          # agreement/exp/cij, layout (c, g)
        cw16 = sp.tile([128, GC], f16)         # layout (c, g)
        mxz = sp.tile([128, G], f32)
        zi = sp.tile([128, G], f32)
        sP = sp.tile([128, CO], f32)           # partial s; [0:32] = s/v
        vgc = sp.tile([128, CO * 2], f16)      # v expanded over g-pair
        sq = sp.tile([B, C], f32)
        tA = sp.tile([B, C], f32)
        tB = sp.tile([B, C], f32)

        u_fg = u[:].rearrange("p (f g) -> p f g", f=CO, g=G)
        u_cog = u[:].rearrange("p (c o g) -> p c o g", c=C, o=O, g=G)
        a_cg = a_e[:].rearrange("p (c g) -> p c g", c=C, g=G)

        # ---------------- production: u_hat matmuls ----------------
        for hg in range(2 * G):
            g, h = hg // 2, hg % 2
            wt = wp.tile([128, CO // 2], f32, tag="wst")
            nc.sync.dma_start(wt[:], w_d[:, hg * (CO // 2):(hg + 1) * (CO // 2)])
            for nch2 in range(2):
                nch = 2 * h + nch2
                ps = pp.tile([128, 512], f32, tag="pp")
                for kg in range(4):
                    nc.tensor.matmul(
                        ps[32 * kg:32 * kg + 32, :],
                        x_sb[32 * kg:32 * kg + 32, g * B:(g + 1) * B],
                        wt[32 * kg:32 * kg + 32, 512 * nch2:512 * (nch2 + 1)],
                        start=True, stop=True,
                        tile_position=(32 * kg, 32 * kg))
                # scatter into u (stride G) on the scalar engine
                nc.scalar.copy(u_fg[:, 512 * nch:512 * (nch + 1), g], ps[:])

        # ---------------- iter-0 s: s0 = sum_k u / C ----------------
        NCHK = 16
        CW = CO // NCHK  # 128
        for j in range(NCHK):
            tree = wkp.tile([128, 4096], f16, tag="work")
            src = u_fg[:, j * CW:(j + 1) * CW, :]
            _tree_last(nc, tree, src, CW, G, sP[:, j * CW:(j + 1) * CW])

        def kg_fold_scale_ar(scaled, tag):
            # fold 4 kg partition groups via PE: s[b,co] = sum_p sel[p,b]*sP[p,co]
            lhs = sel_sb[:, B:2 * B] if scaled else sel_sb[:, 0:B]
            arin = dp.tile([B, CO], f32, tag=f"arin{tag}")
            arout = dp.tile([B, CO], f32, tag=f"arout{tag}")
            for q in range(4):
                fp = pp.tile([B, 512], f32, tag="fold")
                nc.tensor.matmul(fp[:], lhs, sP[:, 512 * q:512 * (q + 1)],
                                 start=True, stop=True)
                nc.vector.tensor_copy(sP[0:32, 512 * q:512 * (q + 1)], fp[:])
            nc.sync.dma_start(arin[:], sP[0:32, :])
            nc.gpsimd.collective_compute(
                "AllReduce", add,
                ins=[arin[:]], outs=[arout[:]],
                replica_groups=[list(range(NCORES))])
            nc.sync.dma_start(sP[0:32, :], arout[:])

        def squash():
            # sP[0:32] holds allreduced s [B, CO]; in-place -> v
            sv = sP[0:32, :]
            wk = wkp.tile([128, 4096], f16, tag="work")
            scr = wk[:].bitcast(f32)[0:32, 0:CO]
            nc.vector.tensor_tensor(scr, sv, sv, mult)
            s2 = scr.rearrange("p (c o) -> p c o", c=C, o=O)
            nc.vector.tensor_reduce(sq[:], s2, axis=mybir.AxisListType.X, op=add)
            nc.vector.tensor_scalar_add(tA[:], sq[:], 1.0)
            nc.vector.reciprocal(tA[:], tA[:])
            nc.vector.tensor_tensor(tA[:], sq[:], tA[:], mult)   # sq/(1+sq)
            nc.vector.tensor_scalar_add(tB[:], sq[:], 1e-8)
            nc.scalar.sqrt(tB[:], tB[:])
            nc.vector.reciprocal(tB[:], tB[:])
            nc.vector.tensor_tensor(tA[:], tA[:], tB[:], mult)   # full factor
            fbc = tA[:].unsqueeze(2).broadcast_to([B, C, O])
            svv = sv.rearrange("p (c o) -> p c o", c=C, o=O)
            nc.vector.tensor_tensor(svv, svv, fbc, mult)

        def build_vgc():
            # expand v [32,(c,o)] fp32 -> vgc [128,(c,o,2)] fp16
            src = sP[0:32, :].rearrange("p (c o) -> p c o", c=C, o=O) \
                .unsqueeze(3).broadcast_to([B, C, O, 2])
            dst = vgc[0:32, :].rearrange("p (c o g) -> p c o g", c=C, o=O, g=2)
            nc.scalar.copy(dst, src)
            for kg in range(1, 4):
                nc.sync.dma_start(vgc[32 * kg:32 * kg + 32, :], vgc[0:32, :])

        kg_fold_scale_ar(True, "0")
        squash()
        build_vgc()

        vgc4 = vgc[:].rearrange("p (c o g) -> p c o g", c=C, o=O, g=2)

        # ---------------- routing iterations ----------------
        for it in (0, 1):
            # agreement a[p,(c,g)] = sum_o u * v, in g-chunks of 2
            for j in range(G // 2):
                am = wkp.tile([128, 4096], f16, tag="work")
                am4 = am[:].rearrange("p (c o g) -> p c o g", c=C, o=O, g=2)
                nc.vector.tensor_tensor(
                    am4, u_cog[:, :, :, 2 * j:2 * j + 2], vgc4, mult)
                tree = wkp.tile([128, 4096], f16, tag="work")
                _tree_mid(nc, tree, am4, C, O, 2, a_cg[:, :, 2 * j:2 * j + 2])

            # b update
            if it == 0:
                nc.vector.tensor_copy(bij[:], a_e[:])
            else:
                nc.vector.tensor_tensor(bij[:], bij[:], a_e[:], add)

            # softmax over c: bij layout (c,g); views [p, g, c]
            bv = bij[:].rearrange("p (c g) -> p g c", c=C, g=G)
            nc.vector.tensor_reduce(mxz[:], bv, axis=mybir.AxisListType.X, op=mx_op)
            mbc = mxz[:].unsqueeze(2).broadcast_to([128, G, C])
            ev = a_e[:].rearrange("p (c g) -> p g c", c=C, g=G)
            nc.vector.tensor_tensor(ev, bv, mbc, sub)
            nc.scalar.activation(a_e[:], a_e[:], mybir.ActivationFunctionType.Exp)
            nc.vector.tensor_reduce(mxz[:], ev, axis=mybir.AxisListType.X, op=add)
            nc.vector.reciprocal(zi[:], mxz[:])
            zbc = zi[:].unsqueeze(2).broadcast_to([128, G, C])
            cv16 = cw16[:].rearrange("p (c g) -> p g c", c=C, g=G)
            nc.vector.tensor_tensor(cv16, ev, zbc, mult)
            if it == 1:
                # final c_ij output (fp32, (c,g) layout), then DMA out
                nc.vector.tensor_tensor(ev, ev, zbc, mult)
                nc.sync.dma_start(cij_d, a_e[:])

            # s pass: sP[p,(c,o)] = sum_g cw16 * u   (c-chunks of 4)
            cv = cw16[:].rearrange("p (c g) -> p c g", c=C, g=G)
            for j in range(NCHK):
                sm = wkp.tile([128, 4096], f16, tag="work")
                sm4 = sm[:].rearrange("p (c o g) -> p c o g", c=4, o=O, g=G)
                cbc = cv[:, 4 * j:4 * j + 4, :].unsqueeze(2) \
                    .broadcast_to([128, 4, O, G])
                nc.vector.tensor_tensor(
                    sm4, u_cog[:, 4 * j:4 * j + 4, :, :], cbc, mult)
                tree = wkp.tile([128, 4096], f16, tag="work")
                smv = sm[:].rearrange("p (m g) -> p m g", m=CW, g=G)
                _tree_last(nc, tree, smv, CW, G, sP[:, j * CW:(j + 1) * CW])

            kg_fold_scale_ar(False, str(it + 1))
            squash()
            if it == 0:
                build_vgc()
            else:
                nc.sync.dma_start(v_d, sP[0:32, :])


_NC_CACHE = {}


def _build():
    if "nc" in _NC_CACHE:
        return _NC_CACHE["nc"]
    nc = Bacc("TRN2", target_bir_lowering=False, debug=False,
              num_devices=NCORES)
    xT_d = nc.dram_tensor("xT", [128, G * B], f32, kind="ExternalInput").ap()
    sel_d = nc.dram_tensor("sel", [128, 2 * B], f32, kind="ExternalInput").ap()
    w_d = nc.dram_tensor("w", [128, FREE], f32, kind="ExternalInput").ap()
    v_d = nc.dram_tensor("v_out", [B, CO], f32, kind="ExternalOutput").ap()
    cij_d = nc.dram_tensor("cij_out", [128, GC], f32, kind="ExternalOutput").ap()
    with tile.TileContext(nc) as tc:
        _kernel_body(nc, tc, xT_d, sel_d, w_d, v_d, cij_d)
    nc.compile()
    _NC_CACHE["nc"] = nc
    return nc


def _shard_inputs(x, W):
    """Host-side shard + layout. Returns in_maps for the 8 cores."""
    in_maps = []
    for r in range(NCORES):
        k0 = r * KSH
        Wr = W[k0:k0 + KSH]                       # [128, C, O, I]
        # -> [(kg, i), (g, c, o)]
        Wr = Wr.reshape(G, 4, C, O, I).transpose(1, 4, 0, 2, 3)
        w_host = np.ascontiguousarray(Wr.reshape(128, FREE), dtype=np.float32)
        xr = x[:, k0:k0 + KSH, :]                 # [B, 128, I]
        xr = xr.reshape(B, G, 4, I).transpose(2, 3, 1, 0)   # [kg, i, g, b]
        x_host = np.ascontiguousarray(xr.reshape(128, G * B), dtype=np.float32)
        sel = np.zeros((128, 2 * B), dtype=np.float32)
        for kg in range(4):
            sel[kg * B:(kg + 1) * B, 0:B] = np.eye(B, dtype=np.float32)
            sel[kg * B:(kg + 1) * B, B:2 * B] = np.eye(B, dtype=np.float32) / C
        in_maps.append({"xT": x_host, "w": w_host, "sel": sel})
    return in_maps


def kernel(x, W, _profile=False):
    x = np.asarray(x, dtype=np.float32)
    W = np.asarray(W, dtype=np.float32)
    nc = _build()
    in_maps = _shard_inputs(x, W)
    res = bass_utils.run_bass_kernel_spmd(
        nc, in_maps, core_ids=list(range(NCORES)), trace=_profile)
    v = res.results[0]["v_out"].reshape(B, C, O)
    cs = []
    for r in range(NCORES):
        cr = res.results[r]["cij_out"].reshape(4, B, C, G)  # [kg, b, c, g]
        cs.append(cr.transpose(1, 3, 0, 2).reshape(B, KSH, C))  # [b,(g,kg),c]
    c_ij = np.concatenate(cs, axis=1)             # [B, K, C]
    if _profile:
        return (v, c_ij), res
    return v, c_ij


# revision 14
# speedup vs baseline: 1.0646x; 1.0471x over previous
"""CapsNet ClassCaps (dynamic routing) Trainium2 kernel, 8-core SPMD.

Problem: u_hat[b,k,c,o] = sum_i W[k,c,o,i] * x[b,k,i]; 3 routing iterations
(softmax over classes, weighted sum over capsules, squash, agreement).
B=32, K=1024, C=64, O=32, I=32.

Sharding: K split 8 ways (128 capsules/core). Each core:
  - streams its W shard [128k, 64c, 32o, 32i] (host pre-laid-out as
    [(k%4)*32+i, (k//4)*2048 + c*32 + o], fp32),
  - computes u_hat with fp32 PE matmuls packed 4-per-array via tile_position
    (contraction i=32, 4 capsules concurrently on diagonal 32x32 tiles),
  - keeps u_hat SBUF-resident in fp16, layout [p=(k%4,b), f=(k//4,c,o)],
  - runs routing on DVE (fused via fp16 tree reductions), with one 256KB
    AllReduce of the partial s_j per routing step (3 total),
  - emits its c_ij shard; v_j is replicated (taken from core 0).
"""
import numpy as np
import concourse.bass as bass
import concourse.mybir as mybir
import concourse.tile as tile
from concourse import bass_utils
from concourse.bacc import Bacc

dt = mybir.dt
f32, f16 = dt.float32, dt.float16

B, K, C, O, I = 32, 1024, 64, 32, 32
NCORES = 8
KSH = K // NCORES          # 128 capsules per core
G = KSH // 4               # 32 k-groups of 4
CO = C * O                 # 2048
FREE = G * CO              # 65536
GC = G * C                 # 2048

_OFF = [0, 2048, 3072, 3584]       # tree tile level offsets (fp16 elements)


def _tree_last(nc, tree, src3, pre, red, final_out_ap):
    """sum over innermost axis of [p, pre, red] (fp16, contiguous views).
    Final add writes fp32 `final_out_ap` [p, pre]."""
    add = mybir.AluOpType.add
    cur, level = src3, 0
    while red > 2:
        half = red // 2
        out = tree[:, _OFF[level]:_OFF[level] + pre * half] \
            .rearrange("p (m r) -> p m r", m=pre, r=half)
        nc.vector.tensor_tensor(out, cur[:, :, :half], cur[:, :, half:red], add)
        cur, red, level = out, half, level + 1
    nc.vector.tensor_tensor(final_out_ap, cur[:, :, 0], cur[:, :, 1], add)


def _tree_mid(nc, tree, src4, a, red, bk, final_out_ap):
    """sum over middle axis of [p, a, red, bk] (fp16). Final add writes fp32
    `final_out_ap` [p, a, bk]."""
    add = mybir.AluOpType.add
    cur, level = src4, 0
    while red > 2:
        half = red // 2
        out = tree[:, _OFF[level]:_OFF[level] + a * half * bk] \
            .rearrange("p (m r n) -> p m r n", m=a, r=half, n=bk)
        nc.vector.tensor_tensor(out, cur[:, :, :half, :], cur[:, :, half:red, :], add)
        cur, red, level = out, half, level + 1
    nc.vector.tensor_tensor(final_out_ap, cur[:, :, 0, :], cur[:, :, 1, :], add)


def _kernel_body(nc, tc, xT_d, sel_d, sel16_d, w_d, v_d, cij_d):
    add = mybir.AluOpType.add
    mult = mybir.AluOpType.mult
    sub = mybir.AluOpType.subtract
    mx_op = mybir.AluOpType.max

    with tc.tile_pool(name="u", bufs=1) as up, \
         tc.tile_pool(name="x", bufs=1) as xp, \
         tc.tile_pool(name="wst", bufs=2) as wp, \
         tc.tile_pool(name="work", bufs=2) as wkp, \
         tc.tile_pool(name="small", bufs=1) as sp, \
         tc.tile_pool(name="ps", bufs=3, space="PSUM") as pp, \
         tc.tile_pool(name="ps1", bufs=1, space="PSUM") as pq, \
         tc.tile_pool(name="dram", bufs=1, space="DRAM") as dp:

        u = up.tile([128, FREE], f16)          # layout [p, (c, o, g)]
        x_sb = xp.tile([128, G * B], f32)
        sel_sb = xp.tile([128, 2 * B], f32)
        sel16_sb = xp.tile([128, B], f16)
        nc.sync.dma_start(x_sb[:], xT_d)
        nc.sync.dma_start(sel_sb[:], sel_d)
        nc.sync.dma_start(sel16_sb[:], sel16_d)

        bij = sp.tile([128, GC], f32)          # layout (c, g)
        a_e = sp.tile([128, GC], f32)          # agreement/exp/cij, layout (c, g)
        cw16 = sp.tile([128, GC], f16)         # layout (c, g)
        mxz = sp.tile([128, G], f32)
        zi = sp.tile([128, G], f32)
        sP = sp.tile([128, CO], f32)           # partial s; [0:32] = s/v
        vgc = sp.tile([128, CO * 2], f16)      # v expanded over g-pair
        sq = sp.tile([B, C], f32)
        tA = sp.tile([B, C], f32)
        tB = sp.tile([B, C], f32)

        u_fg = u[:].rearrange("p (f g) -> p f g", f=CO, g=G)
        u_cog = u[:].rearrange("p (c o g) -> p c o g", c=C, o=O, g=G)
        a_cg = a_e[:].rearrange("p (c g) -> p c g", c=C, g=G)

        # ---------------- production: u_hat matmuls + s0 accumulation ------
        s0ps = []
        for q in range(4):
            s0t = pq.tile([B, 512], f32, tag=f"s0ps{q}")
            s0ps.append(s0t)
        for hg in range(2 * G):
            g, h = hg // 2, hg % 2
            wt = wp.tile([128, CO // 2], f32, tag="wst")
            nc.sync.dma_start(wt[:], w_d[:, hg * (CO // 2):(hg + 1) * (CO // 2)])
            for nch2 in range(2):
                nch = 2 * h + nch2
                ps = pp.tile([128, 512], f32, tag="pp")
                for kg in range(4):
                    nc.tensor.matmul(
                        ps[32 * kg:32 * kg + 32, :],
                        x_sb[32 * kg:32 * kg + 32, g * B:(g + 1) * B],
                        wt[32 * kg:32 * kg + 32, 512 * nch2:512 * (nch2 + 1)],
                        start=True, stop=True,
                        tile_position=(32 * kg, 32 * kg))
                # scatter into u (stride G) on the scalar engine
                uslice = u_fg[:, 512 * nch:512 * (nch + 1), g]
                nc.scalar.copy(uslice, ps[:])
                # s0 partial: accumulate sel16.T @ u_chunk into psum bank nch
                nc.tensor.matmul(
                    s0ps[nch][:], sel16_sb[:], uslice,
                    start=(g == 0), stop=(g == G - 1))

        # ---------------- iter-0 s: copy accumulated s0 to sbuf -----------
        NCHK = 16
        CW = CO // NCHK  # 128
        for q in range(4):
            nc.vector.tensor_copy(sP[0:32, 512 * q:512 * (q + 1)], s0ps[q][:])

        def kg_fold_scale_ar(fold, tag):
            # fold 4 kg partition groups via PE: s[b,co] = sum_p sel[p,b]*sP[p,co]
            arin = dp.tile([B, CO], f32, tag=f"arin{tag}")
            arout = dp.tile([B, CO], f32, tag=f"arout{tag}")
            if fold:
                lhs = sel_sb[:, 0:B]
                for q in range(4):
                    fp = pq.tile([B, 512], f32, tag="fold")
                    nc.tensor.matmul(fp[:], lhs, sP[:, 512 * q:512 * (q + 1)],
                                     start=True, stop=True)
                    nc.vector.tensor_copy(sP[0:32, 512 * q:512 * (q + 1)], fp[:])
            nc.sync.dma_start(arin[:], sP[0:32, :])
            nc.gpsimd.collective_compute(
                "AllReduce", add,
                ins=[arin[:]], outs=[arout[:]],
                replica_groups=[list(range(NCORES))])
            nc.sync.dma_start(sP[0:32, :], arout[:])

        def squash():
            # sP[0:32] holds allreduced s [B, CO]; in-place -> v
            sv = sP[0:32, :]
            wk = wkp.tile([128, 4096], f16, tag="work")
            scr = wk[:].bitcast(f32)[0:32, 0:CO]
            nc.vector.tensor_tensor(scr, sv, sv, mult)
            s2 = scr.rearrange("p (c o) -> p c o", c=C, o=O)
            nc.vector.tensor_reduce(sq[:], s2, axis=mybir.AxisListType.X, op=add)
            nc.vector.tensor_scalar_add(tA[:], sq[:], 1.0)
            nc.vector.reciprocal(tA[:], tA[:])
            nc.vector.tensor_tensor(tA[:], sq[:], tA[:], mult)   # sq/(1+sq)
            nc.vector.tensor_scalar_add(tB[:], sq[:], 1e-8)
            nc.scalar.sqrt(tB[:], tB[:])
            nc.vector.reciprocal(tB[:], tB[:])
            nc.vector.tensor_tensor(tA[:], tA[:], tB[:], mult)   # full factor
            fbc = tA[:].unsqueeze(2).broadcast_to([B, C, O])
            svv = sv.rearrange("p (c o) -> p c o", c=C, o=O)
            nc.vector.tensor_tensor(svv, svv, fbc, mult)

        def build_vgc():
            # expand v [32,(c,o)] fp32 -> vgc [128,(c,o,2)] fp16
            src = sP[0:32, :].rearrange("p (c o) -> p c o", c=C, o=O) \
                .unsqueeze(3).broadcast_to([B, C, O, 2])
            dst = vgc[0:32, :].rearrange("p (c o g) -> p c o g", c=C, o=O, g=2)
            nc.scalar.copy(dst, src)
            for kg in range(1, 4):
                nc.sync.dma_start(vgc[32 * kg:32 * kg + 32, :], vgc[0:32, :])

        kg_fold_scale_ar(False, "0")
        squash()
        build_vgc()

        vgc4 = vgc[:].rearrange("p (c o g) -> p c o g", c=C, o=O, g=2)

        # ---------------- routing iterations ----------------
        for it in (0, 1):
            # agreement a[p,(c,g)] = sum_o u * v, in g-chunks of 2
            for j in range(G // 2):
                am = wkp.tile([128, 4096], f16, tag="work")
                am4 = am[:].rearrange("p (c o g) -> p c o g", c=C, o=O, g=2)
                nc.vector.tensor_tensor(
                    am4, u_cog[:, :, :, 2 * j:2 * j + 2], vgc4, mult)
                tree = wkp.tile([128, 4096], f16, tag="work")
                _tree_mid(nc, tree, am4, C, O, 2, a_cg[:, :, 2 * j:2 * j + 2])

            # b update
            if it == 0:
                nc.vector.tensor_copy(bij[:], a_e[:])
            else:
                nc.vector.tensor_tensor(bij[:], bij[:], a_e[:], add)

            # softmax over c: bij layout (c,g); views [p, g, c]
            bv = bij[:].rearrange("p (c g) -> p g c", c=C, g=G)
            nc.vector.tensor_reduce(mxz[:], bv, axis=mybir.AxisListType.X, op=mx_op)
            mbc = mxz[:].unsqueeze(2).broadcast_to([128, G, C])
            ev = a_e[:].rearrange("p (c g) -> p g c", c=C, g=G)
            nc.vector.tensor_tensor(ev, bv, mbc, sub)
            nc.scalar.activation(a_e[:], a_e[:], mybir.ActivationFunctionType.Exp)
            nc.vector.tensor_reduce(mxz[:], ev, axis=mybir.AxisListType.X, op=add)
            nc.vector.reciprocal(zi[:], mxz[:])
            zbc = zi[:].unsqueeze(2).broadcast_to([128, G, C])
            cv16 = cw16[:].rearrange("p (c g) -> p g c", c=C, g=G)
            nc.vector.tensor_tensor(cv16, ev, zbc, mult)
            if it == 1:
                # final c_ij output (fp32, (c,g) layout), then DMA out
                nc.vector.tensor_tensor(ev, ev, zbc, mult)
                nc.sync.dma_start(cij_d, a_e[:])

            # s pass: sP[p,(c,o)] = sum_g cw16 * u   (c-chunks of 4)
            cv = cw16[:].rearrange("p (c g) -> p c g", c=C, g=G)
            for j in range(NCHK):
                sm = wkp.tile([128, 4096], f16, tag="work")
                sm4 = sm[:].rearrange("p (c o g) -> p c o g", c=4, o=O, g=G)
                cbc = cv[:, 4 * j:4 * j + 4, :].unsqueeze(2) \
                    .broadcast_to([128, 4, O, G])
                nc.vector.tensor_tensor(
                    sm4, u_cog[:, 4 * j:4 * j + 4, :, :], cbc, mult)
                tree = wkp.tile([128, 4096], f16, tag="work")
                smv = sm[:].rearrange("p (m g) -> p m g", m=CW, g=G)
                _tree_last(nc, tree, smv, CW, G, sP[:, j * CW:(j + 1) * CW])

            kg_fold_scale_ar(True, str(it + 1))
            squash()
            if it == 0:
                build_vgc()
            else:
                nc.sync.dma_start(v_d, sP[0:32, :])


_NC_CACHE = {}


def _build():
    if "nc" in _NC_CACHE:
        return _NC_CACHE["nc"]
    nc = Bacc("TRN2", target_bir_lowering=False, debug=False,
              num_devices=NCORES)
    xT_d = nc.dram_tensor("xT", [128, G * B], f32, kind="ExternalInput").ap()
    sel_d = nc.dram_tensor("sel", [128, 2 * B], f32, kind="ExternalInput").ap()
    sel16_d = nc.dram_tensor("sel16", [128, B], f16, kind="ExternalInput").ap()
    w_d = nc.dram_tensor("w", [128, FREE], f32, kind="ExternalInput").ap()
    v_d = nc.dram_tensor("v_out", [B, CO], f32, kind="ExternalOutput").ap()
    cij_d = nc.dram_tensor("cij_out", [128, GC], f32, kind="ExternalOutput").ap()
    with tile.TileContext(nc) as tc:
        _kernel_body(nc, tc, xT_d, sel_d, sel16_d, w_d, v_d, cij_d)
    nc.compile()
    _NC_CACHE["nc"] = nc
    return nc


def _shard_inputs(x, W):
    """Host-side shard + layout. Returns in_maps for the 8 cores."""
    in_maps = []
    for r in range(NCORES):
        k0 = r * KSH
        Wr = W[k0:k0 + KSH]                       # [128, C, O, I]
        # -> [(kg, i), (g, c, o)]
        Wr = Wr.reshape(G, 4, C, O, I).transpose(1, 4, 0, 2, 3)
        w_host = np.ascontiguousarray(Wr.reshape(128, FREE), dtype=np.float32)
        xr = x[:, k0:k0 + KSH, :]                 # [B, 128, I]
        xr = xr.reshape(B, G, 4, I).transpose(2, 3, 1, 0)   # [kg, i, g, b]
        x_host = np.ascontiguousarray(xr.reshape(128, G * B), dtype=np.float32)
        sel = np.zeros((128, 2 * B), dtype=np.float32)
        sel16 = np.zeros((128, B), dtype=np.float16)
        for kg in range(4):
            sel[kg * B:(kg + 1) * B, 0:B] = np.eye(B, dtype=np.float32)
            sel[kg * B:(kg + 1) * B, B:2 * B] = np.eye(B, dtype=np.float32) / C
            sel16[kg * B:(kg + 1) * B, :] = np.eye(B, dtype=np.float16) / C
        in_maps.append({"xT": x_host, "w": w_host, "sel": sel,
                        "sel16": sel16})
    return in_maps


def kernel(x, W, _profile=False):
    x = np.asarray(x, dtype=np.float32)
    W = np.asarray(W, dtype=np.float32)
    nc = _build()
    in_maps = _shard_inputs(x, W)
    res = bass_utils.run_bass_kernel_spmd(
        nc, in_maps, core_ids=list(range(NCORES)), trace=_profile)
    v = res.results[0]["v_out"].reshape(B, C, O)
    cs = []
    for r in range(NCORES):
        cr = res.results[r]["cij_out"].reshape(4, B, C, G)  # [kg, b, c, g]
        cs.append(cr.transpose(1, 3, 0, 2).reshape(B, KSH, C))  # [b,(g,kg),c]
    c_ij = np.concatenate(cs, axis=1)             # [B, K, C]
    if _profile:
        return (v, c_ij), res
    return v, c_ij
